# revision 2
# baseline (speedup 1.0000x reference)
"""Trainium2 Bass kernel: 3x3 stride-1 pad-1 conv2d, NCHW int32 (quantized values).

Strategy
--------
Data-parallel over batch: 32 images -> 8 cores x 4 images. Weights replicated.

Math: inputs are small non-negative ints (x in [0,15], w in [0,14]) so both are
exactly representable in fp8 e4m3. The PE computes fp8 products exactly and
accumulates in fp32 PSUM; max accumulator value 15*14*9*256 < 2^24, so the fp32
accumulation is exact integer arithmetic. We convert int32 -> fp8 on the host,
run the conv as 9 shifted matmuls with DoubleRow perf mode (contracts all 256
input channels in one matmul: K = 128 partitions x 2 k-tiles), and emit int32.

Layout: per core, x is stored padded as [c_lo=128 partitions][img=4][c_hi=2]
[60x60 fp8 plane] with the 56x56 image at rows/cols 1..56 and zeros elsewhere.
With this layout every conv tap (dy,dx) over an 8-row output block is a single
contiguous 480-column slice of the plane, so the matmul rhs AP is
[128, 2, 480] (moving operand 960 fp8 <= 1024 limit, PSUM tile 480 fp32 <= one
bank). Columns w in [56,60) of each PSUM row-block are padding garbage and are
skipped when evacuating.

Weights: [c_lo=128][tap=9][c_hi=2][o=256] fp8; lhsT slice [128, 2, 128] per
(tap, o-chunk).

Output: PSUM fp32 -> (VectorE cast) -> SBUF int32 -> DMA to DRAM [4,256,56,56].
"""

import numpy as np
import ml_dtypes

import concourse.bass as bass
import concourse.bacc as bacc
import concourse.mybir as mybir
import concourse.tile as tile
from concourse import bass_utils

N_CORES = 8
NIMG = 4          # images per core
C = 256           # in channels
O = 256           # out channels
H = W = 56
HP = WP = 60      # padded plane; 60*60 stride is %16 aligned for DoubleRow APs
PLANE = HP * WP   # 3600
RB = 8            # output rows per block
NBLK = H // RB    # 7
NCOLS = RB * WP   # 480 psum columns per block
F8 = ml_dtypes.float8_e4m3

_CACHED_NC = None


def _build_module():
    nc = bacc.Bacc("TRN2", target_bir_lowering=False, debug=False,
                   num_devices=N_CORES)
    xp_d = nc.dram_tensor("xp", [128, NIMG, 2, PLANE], mybir.dt.float8e4,
                          kind="ExternalInput").ap()
    wt_d = nc.dram_tensor("wt", [128, 9, 2, O], mybir.dt.float8e4,
                          kind="ExternalInput").ap()
    y_d = nc.dram_tensor("y", [NIMG, O, H, W], mybir.dt.int32,
                         kind="ExternalOutput").ap()

    with tile.TileContext(nc) as tc:
        with tc.tile_pool(name="w", bufs=1) as wpool, \
             tc.tile_pool(name="x", bufs=2) as xpool, \
             tc.tile_pool(name="ps", bufs=8, space="PSUM") as pspool, \
             tc.tile_pool(name="o", bufs=6) as opool:
            w_sb = wpool.tile([128, 9, 2, O], mybir.dt.float8e4)
            nc.sync.dma_start(w_sb[:], wt_d[:])

            for img in range(NIMG):
                x_sb = xpool.tile([128, 2, PLANE], mybir.dt.float8e4, tag="x")
                nc.sync.dma_start(x_sb[:], xp_d[:, img])
                for oc in range(2):
                    # groups of row-blocks share one LDWEIGHTS per tap and
                    # keep 4+3 PSUM banks in flight
                    for grp in ((0, 1, 2, 3), (4, 5, 6)):
                        pss = [pspool.tile([128, NCOLS], mybir.dt.float32,
                                           tag="ps", name="ps") for _ in grp]
                        for tap in range(9):
                            dy, dx = tap // 3 - 1, tap % 3 - 1
                            lhsT = w_sb[:, tap, :, oc * 128:(oc + 1) * 128]
                            for ps, b in zip(pss, grp):
                                base = (b * RB + 1 + dy) * WP + 1 + dx
                                nc.tensor.matmul(
                                    ps[:], lhsT=lhsT,
                                    rhs=x_sb[:, :, base:base + NCOLS],
                                    start=(tap == 0), stop=(tap == 8),
                                    perf_mode=mybir.MatmulPerfMode.DoubleRow)
                        for ps, b in zip(pss, grp):
                            o_sb = opool.tile([128, RB, W], mybir.dt.int32,
                                              tag="o")
                            src = ps.rearrange("p (r w) -> p r w", w=WP)
                            nc.vector.tensor_copy(o_sb[:], src[:, :, 0:W])
                            nc.sync.dma_start(
                                y_d[img, oc * 128:(oc + 1) * 128,
                                    b * RB:(b + 1) * RB, :],
                                o_sb[:])
    nc.compile()
    return nc


def _get_nc():
    global _CACHED_NC
    if _CACHED_NC is None:
        _CACHED_NC = _build_module()
    return _CACHED_NC


def _prep_inputs(x: np.ndarray, weight: np.ndarray):
    # x: (32, 256, 56, 56) int32 -> per-core padded fp8 planes
    # [core][c_lo=128][img][c_hi][hp][wp]
    xr = x.astype(np.float32).astype(F8).reshape(N_CORES, NIMG, 2, 128, H, W)
    xp_all = np.zeros((N_CORES, 128, NIMG, 2, HP, WP), F8)
    xp_all[:, :, :, :, 1:H + 1, 1:W + 1] = xr.transpose(0, 3, 1, 2, 4, 5)
    xp_all = np.ascontiguousarray(xp_all.reshape(N_CORES, 128, NIMG, 2, PLANE))

    # weight: (O=256, C=256, 3, 3) -> [c_lo=128][tap=9][c_hi=2][o=256]
    wt = weight.astype(np.float32).astype(F8)
    wt = wt.reshape(O, 2, 128, 3, 3).transpose(2, 3, 4, 1, 0)
    wt = np.ascontiguousarray(wt.reshape(128, 9, 2, O))
    return xp_all, wt


def run_on_device(x: np.ndarray, weight: np.ndarray, **run_kwargs):
    """Build in_maps, run the SPMD kernel on 8 cores, return (y, results)."""
    nc = _get_nc()
    xp_all, wt = _prep_inputs(x, weight)
    in_maps = [{"xp": xp_all[c], "wt": wt} for c in range(N_CORES)]
    res = bass_utils.run_bass_kernel_spmd(
        nc, in_maps, core_ids=list(range(N_CORES)), **run_kwargs)
    y = np.concatenate([res.results[c]["y"] for c in range(N_CORES)], axis=0)
    return y, res


def kernel(x: np.ndarray, weight: np.ndarray) -> np.ndarray:
    y, _ = run_on_device(np.asarray(x), np.asarray(weight))
    return y


# revision 3
# speedup vs baseline: 1.0816x; 1.0816x over previous
"""Trainium2 Bass kernel: 3x3 stride-1 pad-1 conv2d, NCHW int32 (quantized values).

Strategy
--------
Data-parallel over batch: 32 images -> 8 cores x 4 images. Weights replicated.

Math: inputs are small non-negative ints (x in [0,15], w in [0,14]) so both are
exactly representable in fp8 e4m3. The PE computes fp8 products exactly and
accumulates in fp32 PSUM; max accumulator value 15*14*9*256 < 2^24, so the fp32
accumulation is exact integer arithmetic. We convert int32 -> fp8 on the host,
run the conv as 9 shifted matmuls with DoubleRow perf mode (contracts all 256
input channels in one matmul: K = 128 partitions x 2 k-tiles), and emit int32.

Layout: per core, x is stored padded as [c_lo=128 partitions][img=4][c_hi=2]
[64x57 fp8 plane] with the 56x56 image at rows 1..56 / cols 1..56 and zeros
elsewhere. Row stride 57 = 1 left pad + 56 pixels: the left zero column of
row r+1 doubles as the right pad of row r, so every conv tap (dy,dx) over an
8-row output block is one contiguous 456-column slice of the plane. The rhs
matmul AP is [128, 2, 456] (moving operand 912 fp8 <= 1024 limit; PSUM tile
456 fp32 <= one bank; c_hi stride 3648 is 16-aligned as DoubleRow requires).
Column w=56 of each block row is padding garbage and is skipped on evacuation.

Weights: [c_lo=128][tap=9][c_hi=2][o=256] fp8; lhsT slice [128, 2, 128] per
(tap, o-chunk). Output: PSUM fp32 -> (VectorE cast) -> SBUF int32 -> DMA out.
Initial DMAs are chunked so the first matmul only waits for the first o-half
of the weights and the top rows of image 0.
"""

import numpy as np
import ml_dtypes

import concourse.bass as bass
import concourse.bacc as bacc
import concourse.mybir as mybir
import concourse.tile as tile
from concourse import bass_utils

N_CORES = 8
NIMG = 4          # images per core
C = 256           # in channels
O = 256           # out channels
H = W = 56
WP = 57           # row stride: 1 left-pad col + 56 pixels
HP = 64           # 1 top halo + 56 rows + 1 bottom halo + margin; 64*57 % 16 == 0
PLANE = HP * WP   # 3648
RB = 8            # output rows per block
NBLK = H // RB    # 7
NCOLS = RB * WP   # 456 psum columns per block
ROWSPLIT = 34     # image-0 DMA split: first chunk covers blocks 0..3 (+halo)
F8 = ml_dtypes.float8_e4m3

_CACHED_NC = None


def _build_module():
    nc = bacc.Bacc("TRN2", target_bir_lowering=False, debug=False,
                   num_devices=N_CORES)
    xp_d = nc.dram_tensor("xp", [128, NIMG, 2, PLANE], mybir.dt.float8e4,
                          kind="ExternalInput").ap()
    wt_d = nc.dram_tensor("wt", [128, 9, 2, O], mybir.dt.float8e4,
                          kind="ExternalInput").ap()
    y_d = nc.dram_tensor("y", [NIMG, O, H, W], mybir.dt.int32,
                         kind="ExternalOutput").ap()

    with tile.TileContext(nc) as tc:
        with tc.tile_pool(name="w", bufs=1) as wpool, \
             tc.tile_pool(name="x", bufs=2) as xpool, \
             tc.tile_pool(name="ps", bufs=8, space="PSUM") as pspool, \
             tc.tile_pool(name="o", bufs=6) as opool:
            w_sb = wpool.tile([128, 9, 2, O], mybir.dt.float8e4)
            # o-chunk 0 of the weights is all the first matmuls need
            nc.sync.dma_start(w_sb[:, :, :, 0:128], wt_d[:, :, :, 0:128])

            x_tiles = []
            for img in range(NIMG):
                x_tiles.append(xpool.tile([128, 2, PLANE], mybir.dt.float8e4,
                                          tag="x", name="x"))

            # image 0 arrives in row chunks so block 0 can start early
            cut = ROWSPLIT * WP
            for ci in range(2):
                nc.sync.dma_start(x_tiles[0][:, ci, 0:cut],
                                  xp_d[:, 0, ci, 0:cut])
            nc.sync.dma_start(w_sb[:, :, :, 128:256], wt_d[:, :, :, 128:256])
            for ci in range(2):
                nc.sync.dma_start(x_tiles[0][:, ci, cut:PLANE],
                                  xp_d[:, 0, ci, cut:PLANE])

            for img in range(NIMG):
                x_sb = x_tiles[img]
                if img + 1 < NIMG:  # prefetch next image, split across queues
                    for ci in range(2):
                        nc.sync.dma_start(x_tiles[img + 1][:, ci],
                                          xp_d[:, img + 1, ci])
                for oc in range(2):
                    for b in range(NBLK):
                        ps = pspool.tile([128, NCOLS], mybir.dt.float32,
                                         tag="ps", name="ps")
                        for tap in range(9):
                            dy, dx = tap // 3 - 1, tap % 3 - 1
                            base = (b * RB + 1 + dy) * WP + 1 + dx
                            nc.tensor.matmul(
                                ps[:],
                                lhsT=w_sb[:, tap, :, oc * 128:(oc + 1) * 128],
                                rhs=x_sb[:, :, base:base + NCOLS],
                                start=(tap == 0), stop=(tap == 8),
                                perf_mode=mybir.MatmulPerfMode.DoubleRow)
                        o_sb = opool.tile([128, RB, W], mybir.dt.int32,
                                          tag="o", name="o")
                        src = ps.rearrange("p (r w) -> p r w", w=WP)
                        nc.vector.tensor_copy(o_sb[:], src[:, :, 0:W])
                        nc.sync.dma_start(
                            y_d[img, oc * 128:(oc + 1) * 128,
                                b * RB:(b + 1) * RB, :],
                            o_sb[:])
    nc.compile()
    return nc


def _get_nc():
    global _CACHED_NC
    if _CACHED_NC is None:
        _CACHED_NC = _build_module()
    return _CACHED_NC


def _prep_inputs(x: np.ndarray, weight: np.ndarray):
    # x: (32, 256, 56, 56) int32 -> per-core padded fp8 planes
    # [core][c_lo=128][img][c_hi][hp][wp]
    xr = x.astype(np.float32).astype(F8).reshape(N_CORES, NIMG, 2, 128, H, W)
    xp_all = np.zeros((N_CORES, 128, NIMG, 2, HP, WP), F8)
    xp_all[:, :, :, :, 1:H + 1, 1:W + 1] = xr.transpose(0, 3, 1, 2, 4, 5)
    xp_all = np.ascontiguousarray(xp_all.reshape(N_CORES, 128, NIMG, 2, PLANE))

    # weight: (O=256, C=256, 3, 3) -> [c_lo=128][tap=9][c_hi=2][o=256]
    wt = weight.astype(np.float32).astype(F8)
    wt = wt.reshape(O, 2, 128, 3, 3).transpose(2, 3, 4, 1, 0)
    wt = np.ascontiguousarray(wt.reshape(128, 9, 2, O))
    return xp_all, wt


def run_on_device(x: np.ndarray, weight: np.ndarray, **run_kwargs):
    """Build in_maps, run the SPMD kernel on 8 cores, return (y, results)."""
    nc = _get_nc()
    xp_all, wt = _prep_inputs(x, weight)
    in_maps = [{"xp": xp_all[c], "wt": wt} for c in range(N_CORES)]
    res = bass_utils.run_bass_kernel_spmd(
        nc, in_maps, core_ids=list(range(N_CORES)), **run_kwargs)
    y = np.concatenate([res.results[c]["y"] for c in range(N_CORES)], axis=0)
    return y, res


def kernel(x: np.ndarray, weight: np.ndarray) -> np.ndarray:
    y, _ = run_on_device(np.asarray(x), np.asarray(weight))
    return y
